# revision 2
# baseline (speedup 1.0000x reference)
"""2D DCT-II (DREAMPlace dct2, N-FFT algorithm) on 8 trn2 NeuronCores.

The reference computes out = A0 @ x @ A1^T where A0/A1 are dense matrices
determined by the expk inputs (the DCT-via-FFT pipeline is linear in x and the
expk twiddle is diagonal):
    A[k, perm[j]] = c_k cos(2*pi*j*k/N) + s_k sin(2*pi*j*k/N),
    perm = makhoul even/odd-reversed permutation, c = expk[:,0], s = expk[:,1].

Sharding: each core computes a 512-column slice of the output,
    out[:, kc] = A0 @ (x @ A1[kc,:]^T)
so there is no cross-core communication.  Stage 1 contracts over x's columns
(n) using x^T tiles as the matmul stationary operand; stage 2 contracts over
rows (r) with A0^T tiles stationary and the SBUF-resident stage-1 result
moving.
"""
import numpy as np

N = 4096
P = 128
NT = N // P          # 32 tiles along a 4096 dim
KC = 512             # output columns per core
NCORES = 8

_NC_CACHE = {}


def _makhoul_perm(n):
    j = np.arange(n)
    return np.where(j < n // 2, 2 * j, 2 * (n - 1 - j) + 1)


def _build_A(expk, n):
    """A s.t. dct1d(v, expk) == v @ A.T for row-vectors v."""
    c = expk[:, 0].astype(np.float64)
    s = expk[:, 1].astype(np.float64)
    k = np.arange(n, dtype=np.int64)
    j = np.arange(n, dtype=np.int64)
    ang = (2.0 * np.pi / n) * ((k[:, None] * j[None, :]) % n).astype(np.float64)
    B = c[:, None] * np.cos(ang) + s[:, None] * np.sin(ang)
    A = np.empty((n, n), dtype=np.float64)
    A[:, _makhoul_perm(n)] = B
    return A.astype(np.float32)


def _build_nc():
    import concourse.bacc as bacc
    import concourse.mybir as mybir
    import concourse.tile as tile

    FP = mybir.dt.float32
    nc = bacc.Bacc("TRN2", target_bir_lowering=False, debug=False,
                   num_devices=NCORES)

    xt = nc.dram_tensor("xt", [N, N], FP, kind="ExternalInput")
    a1tc = nc.dram_tensor("a1tc", [N, KC], FP, kind="ExternalInput")
    a0t = nc.dram_tensor("a0t", [N, N], FP, kind="ExternalInput")
    out = nc.dram_tensor("out", [N, KC], FP, kind="ExternalOutput")

    with tile.TileContext(nc) as tc:
        with tc.tile_pool(name="m1pool", bufs=1) as m1pool:
            # stage-1 result M1^T = x @ A1c^T kept SBUF-resident:
            # [r within tile, (r_tile, k)]
            m1 = m1pool.tile([P, NT, KC], FP)

            with (
                tc.tile_pool(name="a1pool", bufs=1) as a1pool,
                tc.tile_pool(name="xpool", bufs=2) as xpool,
                tc.tile_pool(name="ps1", bufs=8, space="PSUM") as ps1,
            ):
                # A1c^T resident: [n within chunk, (n_chunk, k)]
                a1t = a1pool.tile([P, NT, KC], FP)
                nc.sync.dma_start(
                    a1t[:], a1tc[:].rearrange("(c p) k -> p c k", p=P))

                for rt in range(NT):
                    # x^T block for this r tile: [n within chunk, (n_chunk, r)]
                    xb = xpool.tile([P, NT, P], FP)
                    nc.sync.dma_start(
                        xb[:],
                        xt[:, rt * P:(rt + 1) * P].rearrange(
                            "(c p) r -> p c r", p=P))
                    acc = ps1.tile([P, KC], FP)
                    for ct in range(NT):
                        nc.tensor.matmul(
                            acc[:], xb[:, ct, :], a1t[:, ct, :],
                            start=(ct == 0), stop=(ct == NT - 1))
                    nc.vector.tensor_copy(m1[:, rt, :], acc[:])

            with (
                tc.tile_pool(name="a0pool", bufs=2) as a0pool,
                tc.tile_pool(name="opool", bufs=4) as opool,
                tc.tile_pool(name="ps2", bufs=8, space="PSUM") as ps2,
            ):
                for lt in range(NT):
                    # A0^T block for this l tile: [r within chunk, (r_chunk, l)]
                    ab = a0pool.tile([P, NT, P], FP)
                    nc.sync.dma_start(
                        ab[:],
                        a0t[:, lt * P:(lt + 1) * P].rearrange(
                            "(c p) l -> p c l", p=P))
                    acc = ps2.tile([P, KC], FP)
                    for ct in range(NT):
                        nc.tensor.matmul(
                            acc[:], ab[:, ct, :], m1[:, ct, :],
                            start=(ct == 0), stop=(ct == NT - 1))
                    ot = opool.tile([P, KC], FP)
                    nc.vector.tensor_copy(ot[:], acc[:])
                    nc.sync.dma_start(out[lt * P:(lt + 1) * P, :], ot[:])

    nc.compile()
    return nc


def _get_nc():
    if "nc" not in _NC_CACHE:
        _NC_CACHE["nc"] = _build_nc()
    return _NC_CACHE["nc"]


CHAIN_NAME = "a1tc"  # input whose shape matches the output (for timing chains)


def _make_in_maps(x, expk0, expk1):
    x = np.asarray(x, dtype=np.float32)
    A1 = _build_A(np.asarray(expk1, np.float32), N)
    A0 = _build_A(np.asarray(expk0, np.float32), N)
    xt = np.ascontiguousarray(x.T)
    a0t = np.ascontiguousarray(A0.T)

    in_maps = []
    for c in range(NCORES):
        a1tc = np.ascontiguousarray(A1[c * KC:(c + 1) * KC, :].T)
        in_maps.append({"xt": xt, "a1tc": a1tc, "a0t": a0t})
    return in_maps


def kernel(x, expk0, expk1):
    from concourse.bass_utils import run_bass_kernel_spmd

    in_maps = _make_in_maps(x, expk0, expk1)
    nc = _get_nc()
    res = run_bass_kernel_spmd(nc, in_maps, core_ids=list(range(NCORES)))
    return np.concatenate(
        [res.results[c]["out"] for c in range(NCORES)], axis=1)
